# revision 17
# baseline (speedup 1.0000x reference)
"""Local multi-headed attention (window +/-2) + residual + LayerNorm, Trainium2 Bass kernel.

Sharding: data-parallel over batch. B=8 batch elements -> one per NeuronCore (8 cores).
Each core computes the full sequence for its batch element; no collectives.

v2 layout strategy (vs v1 baseline): all elementwise score/AV work on the DVE is
fused across the 6 d-partition tiles (FD=1536 per op instead of 6x FD=256), with
byte-shifted kc2/vc2 copies so every window tap reads 4B-aligned bf16 (2x DVE mode).
Denominator adds moved gpsimd->DVE (fused); gpsimd keeps only gamma/beta + halo
copies. PSUM drain + residual add + LN mean-sum fused into one scalar_tensor_tensor.

Per-core pipeline per 256-token chunk:
  - x loaded [s,d], PE-transposed to xT [d,s] bf16.
  - Q/K/V projections: W.T @ xT -> qT/kc/vc in [d, s] layout, bf16, PSUM f32,
    ScalarE drain fused with per-partition bias (kc/vc carry +/-2 halo columns).
  - prod_w = qT * shift_w(kc) : 5 fused DVE ops [128,6,256].
  - scores: per d-tile, 5 matmuls with block-ones [128x128] (reduces 64 head dims
    and broadcasts back), exp on ScalarE (scale=1/8), all into ex [128,5,6,256].
  - denominator: 4 fused DVE adds (bf16), reciprocal -> rinv f32.
  - AV: asum accumulation, 9 fused DVE ops; att = asum * rinv.
  - O-projection: attT blocks stationary vs Wo; bias via K=1 ones-row matmul.
  - ypre = (opsum + x) with LN sum via scalar_tensor_tensor accum_out;
    Square+accum on ScalarE; y = (ypre-mu)*rstd*gamma+beta; DMA out.
"""
import os
import sys
import numpy as np

B, S, D = 8, 4096, 768
HEADS = 12
DH = 64
W = 5          # window taps, offsets -2..2
CHUNK = 256    # sequence chunk per inner iteration
NCH = S // CHUNK
DT = D // 128  # 6 partition tiles of d
EPS = 1e-5
N_CORES = 8

_cache = {}


def _build():
    import concourse.bass as bass
    import concourse.tile as tile
    from concourse import bacc, mybir
    from concourse.masks import make_identity

    f32 = mybir.dt.float32
    bf16 = mybir.dt.bfloat16
    AF = mybir.ActivationFunctionType
    ALU = mybir.AluOpType

    nc = bacc.Bacc("TRN2", target_bir_lowering=False, debug=False,
                   num_devices=N_CORES)

    x_ap = nc.dram_tensor("x", [S, D], f32, kind="ExternalInput").ap()
    wq_ap = nc.dram_tensor("Wq", [D, D], f32, kind="ExternalInput").ap()
    bq_ap = nc.dram_tensor("bq", [D], f32, kind="ExternalInput").ap()
    wk_ap = nc.dram_tensor("Wk", [D, D], f32, kind="ExternalInput").ap()
    bk_ap = nc.dram_tensor("bk", [D], f32, kind="ExternalInput").ap()
    wv_ap = nc.dram_tensor("Wv", [D, D], f32, kind="ExternalInput").ap()
    bv_ap = nc.dram_tensor("bv", [D], f32, kind="ExternalInput").ap()
    wo_ap = nc.dram_tensor("Wo", [D, D], f32, kind="ExternalInput").ap()
    bo_ap = nc.dram_tensor("bo", [D], f32, kind="ExternalInput").ap()
    gamma_ap = nc.dram_tensor("gamma", [D], f32, kind="ExternalInput").ap()
    beta_ap = nc.dram_tensor("beta", [D], f32, kind="ExternalInput").ap()
    out_ap = nc.dram_tensor("out", [S, D], f32, kind="ExternalOutput").ap()

    with tile.TileContext(nc) as tc:
        # ---------------- persistent tiles ----------------
        with tc.tile_pool(name="persist", bufs=1) as pp:
            wq_sb = pp.tile([128, DT, D], bf16, tag="wq")
            wk_sb = pp.tile([128, DT, D], bf16, tag="wk")
            wv_sb = pp.tile([128, DT, D], bf16, tag="wv")
            wo_sb = pp.tile([128, DT, D], bf16, tag="wo")
            bqT = pp.tile([128, DT], f32, tag="bqT")
            bkT = pp.tile([128, DT], f32, tag="bkT")
            bvT = pp.tile([128, DT], f32, tag="bvT")
            bo_sb = pp.tile([1, D], f32, tag="bo")
            g_sb = pp.tile([1, D], f32, tag="g")
            be_sb = pp.tile([1, D], f32, tag="be")
            ones_row = pp.tile([1, 128], f32, tag="ones")
            ones_bf = pp.tile([1, 128], bf16, tag="onesbf")
            bo_bf = pp.tile([1, D], bf16, tag="bobf")
            blockones = pp.tile([128, 128], bf16, tag="bones")
            ident = pp.tile([128, 128], f32, tag="ident")
            gb_bc = pp.tile([128, D], bf16, tag="gbbc")
            be_bc = pp.tile([128, D], bf16, tag="bebc")
            kpad = pp.tile([128, DT, 2], bf16, tag="kpad")
            vpad = pp.tile([128, DT, 2], bf16, tag="vpad")

            # weight staging in a transient pool so its SBUF is reclaimed
            with tc.tile_pool(name="wstage", bufs=2) as wst:
                for w_ap, sb in ((wq_ap, wq_sb), (wk_ap, wk_sb),
                                 (wv_ap, wv_sb), (wo_ap, wo_sb)):
                    st = wst.tile([128, DT, D], f32, tag="wstage")
                    nc.sync.dma_start(
                        st[:], w_ap.rearrange("(kt p) n -> p kt n", p=128))
                    nc.vector.tensor_copy(sb[:], st[:])
            nc.sync.dma_start(bqT[:], bq_ap.rearrange("(t p) -> p t", p=128))
            nc.sync.dma_start(bkT[:], bk_ap.rearrange("(t p) -> p t", p=128))
            nc.sync.dma_start(bvT[:], bv_ap.rearrange("(t p) -> p t", p=128))
            nc.sync.dma_start(bo_sb[:], bo_ap[:])
            nc.sync.dma_start(g_sb[:], gamma_ap[:])
            nc.sync.dma_start(be_sb[:], beta_ap[:])

            nc.vector.memset(ones_row[:], 1.0)
            nc.vector.memset(ones_bf[:], 1.0)
            nc.vector.tensor_copy(bo_bf[:], bo_sb[:])
            nc.vector.memset(blockones[:], 0.0)
            nc.vector.memset(blockones[0:64, 0:64], 1.0)
            nc.vector.memset(blockones[64:128, 64:128], 1.0)
            make_identity(nc, ident[:])

            # pad columns: value when attending past the sequence edge is
            # bias-only (0 @ W + b), replicated into 2 halo columns
            nc.vector.memset(kpad[:], 0.0)
            nc.vector.memset(vpad[:], 0.0)
            for dt in range(DT):
                nc.scalar.activation(kpad[:, dt, :], kpad[:, dt, :],
                                     AF.Identity, bias=bkT[:, dt:dt + 1])
                nc.scalar.activation(vpad[:, dt, :], vpad[:, dt, :],
                                     AF.Identity, bias=bvT[:, dt:dt + 1])

            # broadcast gamma/beta across partitions via K=1 matmul
            with tc.tile_pool(name="initps", bufs=1, space="PSUM") as initps:
                for src, dst in ((g_sb, gb_bc), (be_sb, be_bc)):
                    t = initps.tile([128, D], f32, tag="gbps")
                    nc.tensor.matmul(t[:, 0:512], ones_row[:], src[:, 0:512])
                    nc.tensor.matmul(t[:, 512:D], ones_row[:], src[:, 512:D])
                    nc.vector.tensor_copy(dst[:], t[:])

            # ---------------- working pools ----------------
            with tc.tile_pool(name="wpsum", bufs=2, space="PSUM") as wpsum, \
                 tc.tile_pool(name="xpool", bufs=2) as xpool, \
                 tc.tile_pool(name="xrpool", bufs=2) as xrpool, \
                 tc.tile_pool(name="xtpool", bufs=2) as xtpool, \
                 tc.tile_pool(name="qpool", bufs=2) as qpool, \
                 tc.tile_pool(name="kvpool", bufs=4) as kvpool, \
                 tc.tile_pool(name="kv2pool", bufs=1) as kv2pool, \
                 tc.tile_pool(name="prpool", bufs=1) as prpool, \
                 tc.tile_pool(name="expool", bufs=2) as expool, \
                 tc.tile_pool(name="dnpool", bufs=1) as dnpool, \
                 tc.tile_pool(name="ripool", bufs=1) as ripool, \
                 tc.tile_pool(name="avpool", bufs=1) as avpool, \
                 tc.tile_pool(name="atpool", bufs=2) as atpool, \
                 tc.tile_pool(name="ypool", bufs=2) as ypool, \
                 tc.tile_pool(name="stpool", bufs=2) as stpool:

                kc_tiles = [None] * NCH
                vc_tiles = [None] * NCH

                def project(c):
                    """projections for chunk c -> qT (bf16) and kc/vc center cols."""
                    s0 = c * CHUNK
                    x_sb = xpool.tile([128, 2, D], f32, tag="x")
                    nc.sync.dma_start(
                        x_sb[:], x_ap[s0:s0 + CHUNK, :].rearrange(
                            "(st p) d -> p st d", p=128))
                    # transpose to xT bf16 [128, DT, CHUNK]
                    xT = xtpool.tile([128, DT, CHUNK], bf16, tag="xT")
                    for p2 in range(DT // 2):
                        tp = wpsum.tile([128, 2, CHUNK], f32, tag="proj")
                        for d2 in range(2):
                            dt = 2 * p2 + d2
                            for st in range(2):
                                nc.tensor.transpose(
                                    tp[:, d2, st * 128:(st + 1) * 128],
                                    x_sb[:, st, dt * 128:(dt + 1) * 128],
                                    ident[:])
                        nc.scalar.activation(xT[:, 2 * p2:2 * p2 + 2, :], tp[:], AF.Identity)

                    qT = qpool.tile([128, DT, CHUNK], bf16, tag="qT")
                    kc = kvpool.tile([128, DT, CHUNK + 4], bf16, tag="kc", bufs=3)
                    vc = kvpool.tile([128, DT, CHUNK + 4], bf16, tag="vc", bufs=4)
                    kc_tiles[c] = kc
                    vc_tiles[c] = vc
                    for (wsb, bT, dst, off) in ((wq_sb, bqT, qT, None),
                                                (wk_sb, bkT, kc, 2),
                                                (wv_sb, bvT, vc, 2)):
                        for p2 in range(DT // 2):
                            ps = wpsum.tile([128, 2, CHUNK], f32, tag="proj")
                            for d2 in range(2):
                                dt = 2 * p2 + d2
                                for kt in range(DT):
                                    nc.tensor.matmul(
                                        ps[:, d2, :],
                                        wsb[:, kt, dt * 128:(dt + 1) * 128],
                                        xT[:, kt, :],
                                        start=(kt == 0), stop=(kt == DT - 1))
                            for d2 in range(2):
                                dt = 2 * p2 + d2
                                dslice = dst[:, dt, :] if off is None \
                                    else dst[:, dt, 2:2 + CHUNK]
                                nc.scalar.activation(dslice, ps[:, d2, :],
                                                     AF.Identity,
                                                     bias=bT[:, dt:dt + 1])
                    # halo fills (gpsimd: tiny strided copies, keeps DVE free)
                    if c > 0:
                        for big_prev, big_cur in ((kc_tiles[c - 1], kc),
                                                  (vc_tiles[c - 1], vc)):
                            nc.gpsimd.tensor_copy(big_cur[:, :, 0:2],
                                                  big_prev[:, :, CHUNK:CHUNK + 2])
                            nc.gpsimd.tensor_copy(big_prev[:, :, CHUNK + 2:CHUNK + 4],
                                                  big_cur[:, :, 2:4])
                    if c == 0:
                        nc.gpsimd.tensor_copy(kc[:, :, 0:2], kpad[:])
                        nc.gpsimd.tensor_copy(vc[:, :, 0:2], vpad[:])
                    if c == NCH - 1:
                        nc.gpsimd.tensor_copy(kc[:, :, CHUNK + 2:CHUNK + 4], kpad[:])
                        nc.gpsimd.tensor_copy(vc[:, :, CHUNK + 2:CHUNK + 4], vpad[:])
                    return x_sb, qT

                def phaseA(c, qT):
                    """tap products + scores + exp for chunk c."""
                    kc = kc_tiles[c]
                    kc2 = kv2pool.tile([128, DT, CHUNK + 2], bf16, tag="kc2")
                    nc.scalar.activation(kc2[:], kc[:, :, 1:CHUNK + 3],
                                         AF.Identity)
                    prod = prpool.tile([128, W, DT, CHUNK], bf16, tag="prod")
                    ktap = [(kc, 0), (kc2, 0), (kc, 2), (kc2, 2), (kc, 4)]
                    for w, (src, o) in enumerate(ktap):
                        nc.vector.tensor_tensor(
                            prod[:, w], qT[:], src[:, :, o:o + CHUNK], ALU.mult)
                    ex = expool.tile([128, W, DT, CHUNK], bf16, tag="ex")
                    for dt in range(DT):
                        sc = wpsum.tile([128, W, CHUNK], f32, tag="big")
                        for w in range(W):
                            nc.tensor.matmul(sc[:, w, :], blockones[:],
                                             prod[:, w, dt])
                        nc.scalar.activation(ex[:, :, dt], sc[:], AF.Exp,
                                             scale=0.125)
                    return ex

                def phaseB1(c, ex):
                    """softmax denominator + AV for chunk c -> att."""
                    vc = vc_tiles[c]
                    vc2 = kv2pool.tile([128, DT, CHUNK + 2], bf16, tag="vc2")
                    nc.scalar.activation(vc2[:], vc[:, :, 1:CHUNK + 3],
                                         AF.Identity)
                    # denominator: 3 bf16 adds + final add with f32 output
                    s1 = dnpool.tile([128, DT, CHUNK], bf16, tag="s1")
                    s2 = dnpool.tile([128, DT, CHUNK], bf16, tag="s2")
                    nc.vector.tensor_tensor(s1[:], ex[:, 0], ex[:, 1], ALU.add)
                    nc.vector.tensor_tensor(s2[:], ex[:, 2], ex[:, 3], ALU.add)
                    nc.vector.tensor_tensor(s1[:], s1[:], s2[:], ALU.add)
                    dn = ripool.tile([128, DT, CHUNK], f32, tag="dnf")
                    nc.vector.tensor_tensor(dn[:], s1[:], ex[:, 4], ALU.add)
                    rinv = ripool.tile([128, DT, CHUNK], f32, tag="rinv")
                    nc.vector.reciprocal_approx_fast(rinv[:], dn[:])
                    # AV tree: pair-sums on gpsimd overlap DVE mults
                    vtap = [(vc, 0), (vc2, 0), (vc, 2), (vc2, 2), (vc, 4)]
                    mA = avpool.tile([128, DT, CHUNK], bf16, tag="mA")
                    mB = avpool.tile([128, DT, CHUNK], bf16, tag="mB")
                    a = dnpool.tile([128, DT, CHUNK], bf16, tag="s1")
                    b = dnpool.tile([128, DT, CHUNK], bf16, tag="s2")
                    nc.vector.tensor_tensor(mA[:], ex[:, 0],
                                            vc[:, :, 0:CHUNK], ALU.mult)
                    nc.vector.tensor_tensor(mB[:], ex[:, 1],
                                            vc2[:, :, 0:CHUNK], ALU.mult)
                    nc.vector.tensor_tensor(a[:], mA[:], mB[:], ALU.add)
                    mC = avpool.tile([128, DT, CHUNK], bf16, tag="mA")
                    mD = avpool.tile([128, DT, CHUNK], bf16, tag="mB")
                    nc.vector.tensor_tensor(mC[:], ex[:, 2],
                                            vc[:, :, 2:2 + CHUNK], ALU.mult)
                    nc.vector.tensor_tensor(mD[:], ex[:, 3],
                                            vc2[:, :, 2:2 + CHUNK], ALU.mult)
                    nc.vector.tensor_tensor(b[:], mC[:], mD[:], ALU.add)
                    mE = avpool.tile([128, DT, CHUNK], bf16, tag="mA")
                    nc.vector.tensor_tensor(mE[:], ex[:, 4],
                                            vc[:, :, 4:4 + CHUNK], ALU.mult)
                    nc.vector.tensor_tensor(a[:], a[:], b[:], ALU.add)
                    nc.vector.tensor_tensor(a[:], a[:], mE[:], ALU.add)
                    att = atpool.tile([128, DT, CHUNK], bf16, tag="att")
                    nc.gpsimd.tensor_tensor(att[:], a[:], rinv[:], ALU.mult)
                    # prefetch residual x for phaseB2 (DMA is idle)
                    s0 = c * CHUNK
                    xres = xrpool.tile([128, 2, D], f32, tag="xres")
                    nc.sync.dma_start(
                        xres[:], x_ap[s0:s0 + CHUNK, :].rearrange(
                            "(st p) d -> p st d", p=128))
                    return att, xres

                def phaseB2(c, att, xres):
                    """O-projection + bias + residual + LayerNorm for chunk c."""
                    s0 = c * CHUNK
                    for st in range(2):
                        op = wpsum.tile([128, D], f32, tag="big")
                        for dt in range(DT):
                            a_blk = att[:, dt, st * 128:(st + 1) * 128]
                            nc.tensor.matmul(op[:, 0:512], a_blk,
                                             wo_sb[:, dt, 0:512],
                                             start=(dt == 0), stop=False)
                            nc.tensor.matmul(op[:, 512:D], a_blk,
                                             wo_sb[:, dt, 512:D],
                                             start=(dt == 0), stop=False)
                        nc.tensor.matmul(op[:, 0:512], ones_bf[:],
                                         bo_bf[:, 0:512], start=False, stop=True)
                        nc.tensor.matmul(op[:, 512:D], ones_bf[:],
                                         bo_bf[:, 512:D], start=False, stop=True)
                        # ypre = opsum + x, with LN mean-sum accumulated free
                        stats = stpool.tile([128, 8], f32, tag="stats")
                        ypre = ypool.tile([128, D], f32, tag="ypre")
                        nc.vector.scalar_tensor_tensor(
                            ypre[:], op[:], 1.0, xres[:, st, :],
                            ALU.mult, ALU.add, accum_out=stats[:, 0:1])
                        y2 = ypool.tile([128, D], f32, tag="y2")
                        nc.scalar.activation(y2[:], ypre[:], AF.Square,
                                             accum_out=stats[:, 1:2])
                        # var = (sumsq - sum^2/768)/768 ; rstd = 1/sqrt(var+eps)
                        nc.gpsimd.tensor_tensor(stats[:, 2:3], stats[:, 0:1],
                                                stats[:, 0:1], ALU.mult)
                        nc.gpsimd.tensor_scalar_mul(stats[:, 2:3], stats[:, 2:3],
                                                    -1.0 / D)
                        nc.gpsimd.tensor_tensor(stats[:, 2:3], stats[:, 2:3],
                                                stats[:, 1:2], ALU.add)
                        nc.gpsimd.tensor_scalar(stats[:, 3:4], stats[:, 2:3],
                                                1.0 / D, EPS, ALU.mult, ALU.add)
                        nc.scalar.sqrt(stats[:, 4:5], stats[:, 3:4])
                        nc.vector.reciprocal(stats[:, 5:6], stats[:, 4:5])
                        # negmurstd = -sum/D * rstd
                        nc.gpsimd.tensor_tensor(stats[:, 6:7], stats[:, 0:1],
                                                stats[:, 5:6], ALU.mult)
                        nc.gpsimd.tensor_scalar_mul(stats[:, 6:7], stats[:, 6:7],
                                                    -1.0 / D)
                        y1 = ypool.tile([128, D], f32, tag="y1", bufs=1)
                        nc.scalar.activation(y1[:], ypre[:], AF.Identity,
                                             bias=stats[:, 6:7],
                                             scale=stats[:, 5:6])
                        nc.gpsimd.tensor_tensor(y2[:], y1[:], gb_bc[:], ALU.mult)
                        nc.gpsimd.tensor_tensor(y2[:], y2[:], be_bc[:], ALU.add)
                        nc.sync.dma_start(
                            out_ap[s0 + st * 128: s0 + (st + 1) * 128, :], y2[:])

                # 4-stage software pipeline:
                #   t: project(t) | phaseA(t-1) | phaseB2(t-3) | phaseB1(t-2)
                qts = {}
                exs = {}
                atts = {}
                for t in range(NCH + 3):
                    if t < NCH:
                        _, qts[t] = project(t)
                    if 0 <= t - 2 < NCH:
                        atts[t - 2] = phaseB1(t - 2, exs.pop(t - 2))
                    if 0 <= t - 3 < NCH:
                        phaseB2(t - 3, *atts.pop(t - 3))
                    if 0 <= t - 1 < NCH:
                        exs[t - 1] = phaseA(t - 1, qts.pop(t - 1))

    nc.compile()
    return nc


def kernel(**inputs):
    if "nc" not in _cache:
        _cache["nc"] = _build()
    nc = _cache["nc"]
    from concourse.bass_utils import run_bass_kernel_spmd

    names = ["Wq", "bq", "Wk", "bk", "Wv", "bv", "Wo", "bo", "gamma", "beta"]
    shared = {n: np.ascontiguousarray(np.asarray(inputs[n], dtype=np.float32))
              for n in names}
    x = np.asarray(inputs["x"], dtype=np.float32)
    in_maps = [dict(shared, x=np.ascontiguousarray(x[b])) for b in range(N_CORES)]
    res = run_bass_kernel_spmd(nc, in_maps, core_ids=list(range(N_CORES)))
    out = np.stack([res.results[i]["out"] for i in range(N_CORES)], axis=0)
    return out.astype(np.float32)
